# revision 24
# baseline (speedup 1.0000x reference)
"""Trainium2 Bass kernel for nn_Net_19387482374339.

Net: per-batch-element scalar LSTM (IN=1, HID=1) over SEQ=3 steps, then a
Linear(18 -> 1) over flattened groups of 6 consecutive batch elements.

Strategy (v3):
  - Pure data parallel over 8 NeuronCores (batch split).
  - x cast to fp16 on host, uploaded [T, NP=126, SEQ*F] so each tile is one
    contiguous ~2.25MB DMA. 126 partitions = 21 output groups x 6 members;
    the final Linear is 3 tiny TensorE matmuls into PSUM.
  - ScalarE (ACT) is the throughput wall (1 elem/cyc/lane, dtype-blind), so
    ACT does ONLY transcendentals: t=0 collapses into one shared tanh basis
    tau = tanh(al*x0+be) with h1 ~= a*tau+d, c1 ~= a2*tau+d2 (fitted at
    build time, affine parts folded into downstream scales/biases/linear
    weights). 17-18 ACT ops/tile, reading gate args straight from PSUM.
  - Gate pre-activations z_g = x_t + r_g*(tau|h2) are built by TensorE
    (idle otherwise) as identity+diagonal accumulating matmuls into PSUM
    half-tiles; ACT applies func(w_g*z+b) via its free input affine.
    This removes all scalar_tensor_tensor ops from DVE (STT only has a
    1x-mode uop on TRN2 -- measured 1948ns vs 1030ns for tensor_tensor).
  - DVE keeps only true elementwise products/adds in fp16 2x mode
    (~9 ops/tile) plus PSUM->SBUF output copies, and optionally evaluates
    tanh(c3) as a fitted quadratic/cubic (validated at build time) to
    shave ACT further.
  - Software pipeline interleaves tile k's two LSTM steps with k+1/k+2
    prefetch so ACT's in-order stream never waits on the DVE/TensorE tails.
  - If any numerical guard fails (generic weights), falls back to a
    conservative all-DVE/ACT builder (v2) that is correct for any params.
"""

import numpy as np

N_CORES = 8
B = 12582912
SEQ = 3
Bc = B // N_CORES            # 1,572,864 elements per core
GC = Bc // 6                 # 262,144 output groups per core
NP = 126                     # SBUF partitions used (21 groups of 6)
NQ = 21                      # group blocks
T = 7                        # tiles per core
F = 1792                     # elements per partition per tile
H = F // 2                   # z half width (fits 2 PSUM banks as fp32)
PAD_E = T * NP * F           # 1,580,544 padded elements per core
CHUNKS = [(c0, min(512, F - c0)) for c0 in range(0, F, 512)]
ZCHUNKS = [(c0, min(512, H - c0)) for c0 in range(0, H, 512)]

# variable-F tiling for the v4 path: 6 wide tiles + a small drain tile
FS = [1920] * 6 + [964]
SUMF = sum(FS)                              # 12,484 per partition
XOFF = [3 * sum(FS[:k]) for k in range(len(FS))]
OOFF = [sum(FS[:k]) for k in range(len(FS))]
PAD_E2 = NP * SUMF                          # 1,572,984 elements per core


def _CHK(n):
    return [(c0, min(512, n - c0)) for c0 in range(0, n, 512)]

_CACHE = {}


def _fit_shared_tanh(wi, wf, wg, wo, bi, bg, bo):
    """Fit h1(x) ~= a*tanh(al*x+be)+d and c1(x) ~= a2*tanh(al*x+be)+d2
    (shared inner argument) over x~N(0,1). Pure numpy."""
    xs = np.linspace(-6.2, 6.2, 2401)
    gw = np.exp(-xs * xs / 2)
    wts = gw + 3e-4
    sig = lambda z: 1.0 / (1.0 + np.exp(-z))
    c1x = sig(wi * xs + bi) * np.tanh(wg * xs + bg)
    h1x = sig(wo * xs + bo) * np.tanh(c1x)

    sw = np.sum(wts)
    swy_h = np.sum(wts * h1x)
    swy_c = np.sum(wts * c1x)

    def cost(al, be):
        tau = np.tanh(al * xs + be)
        swt = wts * tau
        s_tt = np.sum(swt * tau)
        s_t = np.sum(swt)
        det = s_tt * sw - s_t * s_t
        if abs(det) < 1e-12:
            return np.inf, None
        tot = 0.0
        prm = []
        for y, swy in ((h1x, swy_h), (c1x, swy_c)):
            sty = np.sum(swt * y)
            a = (sty * sw - s_t * swy) / det
            d = (s_tt * swy - s_t * sty) / det
            r = a * tau + d - y
            tot += np.sum(wts * r * r)
            prm.append((a, d))
        return tot, prm

    best = (np.inf, None, None, None)
    for al in np.linspace(0.05, 1.5, 59):
        for be in np.linspace(-2.5, 2.5, 51):
            c, prm = cost(al, be)
            if c < best[0]:
                best = (c, al, be, prm)
    span_al, span_be = 0.06, 0.12
    for _ in range(6):
        _, al0, be0, _ = best
        for al in np.linspace(al0 - span_al, al0 + span_al, 13):
            for be in np.linspace(be0 - span_be, be0 + span_be, 13):
                c, prm = cost(al, be)
                if c < best[0]:
                    best = (c, al, be, prm)
        span_al /= 4.0
        span_be /= 4.0
    _, al, be, ((a, d), (a2, d2)) = best
    tau = np.tanh(al * xs + be)
    rms_h = np.sqrt(np.average((a * tau + d - h1x) ** 2, weights=gw))
    rms_c = np.sqrt(np.average((a2 * tau + d2 - c1x) ** 2, weights=gw))
    return al, be, a, d, a2, d2, rms_h, rms_c


def _fit_tc3_poly(wkey):
    """Fit tanh(c3) as a low-degree polynomial over the reachable c3 range.
    Returns (coeffs or None). Coeffs qs give tanh(c3) ~= sum qs[i] c3^i."""
    (wi, wf, wg, wo, ui, uf, ug, uo, bi, bf, bg, bo) = wkey
    sig = lambda z: 1.0 / (1.0 + np.exp(-z))
    rng = np.random.default_rng(12345)
    xs = rng.standard_normal((200000, 3))
    ext = np.array([-6.0, 0.0, 6.0])
    grid = np.stack(np.meshgrid(ext, ext, ext), -1).reshape(-1, 3)
    xs = np.concatenate([xs, grid], 0)
    h = np.zeros(len(xs)); c = np.zeros(len(xs))
    for t in range(2):
        xt = xs[:, t]
        i = sig(wi * xt + ui * h + bi); f = sig(wf * xt + uf * h + bf)
        g = np.tanh(wg * xt + ug * h + bg); o = sig(wo * xt + uo * h + bo)
        c = f * c + i * g
        h = o * np.tanh(c)
    xt = xs[:, 2]
    i = sig(wi * xt + ui * h + bi); f = sig(wf * xt + uf * h + bf)
    g = np.tanh(wg * xt + ug * h + bg)
    c3 = f * c + i * g
    lo, hi = c3.min(), c3.max()
    span = hi - lo
    lo -= 0.25 * span + 0.05
    hi += 0.25 * span + 0.05
    gridc = np.linspace(lo, hi, 2001)
    y = np.tanh(gridc)
    for deg in (2, 3):
        ch = np.polynomial.Chebyshev.fit(gridc, y, deg)
        qs = ch.convert(kind=np.polynomial.Polynomial).coef
        err = np.abs(np.polynomial.polynomial.polyval(gridc, qs) - y).max()
        if err < 3e-3:
            return tuple(float(q) for q in qs)
    return None


# --------------------------------------------------------------------------
# v4 builder: exact t0, TensorE gate combines, minimal DVE op counts.
# --------------------------------------------------------------------------
def _build_v4(key):
    (wkey, qs1, drop_f) = key
    (wi, wf, wg, wo, ui, uf, ug, uo, bi, bf, bg, bo) = wkey
    import concourse.bacc as bacc
    import concourse.tile as tile
    from concourse import mybir

    dt = mybir.dt
    AF = mybir.ActivationFunctionType
    ALU = mybir.AluOpType
    F16 = dt.float16

    # gate order (i, g, f, o): lets m = i*g start after the first two gates
    gorder = ((wi, ui, bi, AF.Sigmoid),
              (wg, ug, bg, AF.Tanh),
              (wf, uf, bf, AF.Sigmoid),
              (wo, uo, bo, AF.Sigmoid))

    biases = set()
    for (w_, u_, b_, _fn) in gorder:
        biases.add(float(b_))

    nc = bacc.Bacc("TRN2", target_bir_lowering=False, debug=False)
    for v in sorted(biases):
        if v == 0.0:
            continue
        t = nc.alloc_sbuf_tensor(f"const-user-{v!r}", [128, 1], dt.float32)
        nc.vector.memset(t.ap(), v)
        nc.const_aps.aps[(dt.float32, v)] = t.ap()
    nc.all_engine_barrier()

    xd = nc.declare_dram_parameter("x", [NP, 3 * SUMF], F16, isOutput=False)
    zwd = nc.declare_dram_parameter("zw", [5, NP, NP], F16, isOutput=False)
    wds = [nc.declare_dram_parameter(f"w{t + 1}", [NP, NQ], F16, isOutput=False)
           for t in range(3)]
    outd = nc.declare_dram_parameter("out", [NQ, SUMF], F16, isOutput=True)

    with tile.TileContext(nc) as tc:
        with tc.tile_pool(name="wpool", bufs=1) as wpool, \
             tc.tile_pool(name="sbuf", bufs=2) as pool, \
             tc.tile_pool(name="psum", bufs=2, space="PSUM") as psum_pool:
            st = [dict() for _ in range(T)]
            # tile 0's x0 slice first: it gates the very first ACT op
            F0 = FS[0]
            xt0 = pool.tile([NP, 3 * F0], F16, tag="x", bufs=3, name="x_0")
            nc.sync.dma_start(xt0[:, 0:F0], xd[:, XOFF[0]:XOFF[0] + F0])
            st[0]["x"] = xt0
            zw = []
            for m in range(5):
                w = wpool.tile([NP, NP], F16, tag=f"zw{m}")
                nc.sync.dma_start(w[:], zwd[m])
                zw.append(w)
            wt = []
            for wd in wds:
                w = wpool.tile([NP, NQ], F16, tag=f"w{wd.name}")
                nc.sync.dma_start(w[:], wd[:])
                wt.append(w)

            def DMA(k):
                F = FS[k]
                xo = XOFF[k]
                if k == 0:
                    # x0 slice already issued pre-weights; fetch the rest
                    nc.sync.dma_start(st[0]["x"][:, F:3 * F],
                                      xd[:, xo + F:xo + 3 * F])
                    return
                xt = pool.tile([NP, 3 * F], F16, tag="x", bufs=3, name=f"x_{k}")
                if k == 1:
                    nc.sync.dma_start(xt[:, 0:F], xd[:, xo:xo + F])
                    nc.sync.dma_start(xt[:, F:3 * F], xd[:, xo + F:xo + 3 * F])
                else:
                    nc.sync.dma_start(xt[:], xd[:, xo:xo + 3 * F])
                st[k]["x"] = xt

            def T0A(k):
                F = FS[k]
                x0 = st[k]["x"][:, 0:F]
                i0 = pool.tile([NP, F], F16, tag="i0", bufs=2, name=f"i0_{k}")
                g0 = pool.tile([NP, F], F16, tag="g0", bufs=2, name=f"g0_{k}")
                nc.scalar.activation(i0[:], x0, AF.Sigmoid, bias=float(bi),
                                     scale=float(wi))
                nc.scalar.activation(g0[:], x0, AF.Tanh, bias=float(bg),
                                     scale=float(wg))
                st[k]["i0"] = i0
                st[k]["g0"] = g0

            def T0B(k):
                F = FS[k]
                c1 = pool.tile([NP, F], F16, tag="c1", bufs=3, name=f"c1_{k}")
                nc.vector.tensor_tensor(c1[:], st[k]["i0"][:], st[k]["g0"][:],
                                        ALU.mult)
                st[k]["c1"] = c1

            def T0C(k):
                F = FS[k]
                x0 = st[k]["x"][:, 0:F]
                o0 = pool.tile([NP, F], F16, tag="o0", bufs=2, name=f"o0_{k}")
                nc.scalar.activation(o0[:], x0, AF.Sigmoid, bias=float(bo),
                                     scale=float(wo))
                st[k]["o0"] = o0

            def T0D(k):
                F = FS[k]
                tc1 = pool.tile([NP, F], F16, tag="tc1", bufs=2, name=f"tc1_{k}")
                c1 = st[k]["c1"]
                if qs1 is None or k == 0:
                    nc.scalar.activation(tc1[:], c1[:], AF.Tanh,
                                         bias=0.0, scale=1.0)
                else:
                    # odd poly: tanh(c) ~= c*(k0 + k1 c^2 + k2 c^4)
                    k0, k1, k2 = qs1
                    sq = pool.tile([NP, F], F16, tag="sq", bufs=2, name=f"sq_{k}")
                    nc.vector.tensor_tensor(sq[:], c1[:], c1[:], ALU.mult)
                    pB = pool.tile([NP, F], F16, tag="pB", bufs=2, name=f"pB_{k}")
                    nc.vector.tensor_scalar(pB[:], sq[:], float(k2), float(k1),
                                            ALU.mult, ALU.add)
                    pC = pool.tile([NP, F], F16, tag="pC", bufs=2, name=f"pC_{k}")
                    nc.vector.tensor_tensor(pC[:], sq[:], pB[:], ALU.mult)
                    nc.vector.tensor_scalar(pC[:], pC[:], 1.0, float(k0),
                                            ALU.mult, ALU.add)
                    nc.vector.tensor_tensor(tc1[:], c1[:], pC[:], ALU.mult)
                st[k]["tc1"] = tc1

            def T0E(k):
                F = FS[k]
                h1 = pool.tile([NP, F], F16, tag="h1", bufs=3, name=f"h1_{k}")
                nc.vector.tensor_tensor(h1[:], st[k]["o0"][:], st[k]["tc1"][:],
                                        ALU.mult)
                st[k]["h1"] = h1

            def ZGEN(k, step, gi_, half):
                """TensorE: z = x_step + r_g * basis into a PSUM half-tile."""
                F = FS[k]; H = F // 2; ZCHUNKS = _CHK(H)
                src_x = st[k]["x"][:, step * F + half * H: step * F + half * H + H]
                basis = st[k]["h1"] if step == 1 else st[k]["h2"]
                widx = 1 + gi_
                z = psum_pool.tile([NP, H], dt.float32, tag="z", bufs=2,
                                   name=f"z{step}{gi_}h{half}_{k}")
                for (c0, cw) in ZCHUNKS:
                    nc.tensor.matmul(z[:, c0:c0 + cw], zw[0][:],
                                     src_x[:, c0:c0 + cw], start=True, stop=False)
                    nc.tensor.matmul(z[:, c0:c0 + cw], zw[widx][:],
                                     basis[:, half * H + c0: half * H + c0 + cw],
                                     start=False, stop=True)
                return z

            def GREAD(k, step, gi_, half, z):
                F = FS[k]; H = F // 2
                (w_, u_, b_, fn) = gorder[gi_]
                gs = st[k].setdefault(f"g{step}", [None] * 4)
                if gs[gi_] is None:
                    gs[gi_] = pool.tile([NP, F], F16, tag=f"g{gi_}", bufs=2,
                                        name=f"g{step}{gi_}_{k}")
                nc.scalar.activation(gs[gi_][:, half * H: half * H + H], z[:],
                                     fn, bias=float(b_), scale=float(w_))

            def SG(k, step, gi_):
                """DVE combine: s = x_step + (u/w) * basis (fp16 SBUF)."""
                F = FS[k]
                (w_, u_, b_, fn) = gorder[gi_]
                xsl = st[k]["x"][:, step * F:(step + 1) * F]
                basis = st[k]["h1"] if step == 1 else st[k]["h2"]
                s = pool.tile([NP, F], F16, tag=f"s{gi_}", bufs=2,
                              name=f"s{step}{gi_}_{k}")
                nc.vector.tensor_scalar(s[:], basis[:], float(u_ / w_), None,
                                        ALU.mult)
                nc.vector.tensor_tensor(s[:], s[:], xsl, ALU.add)
                st[k][f"s{step}{gi_}"] = s

            def GDIRECT(k, step, gi_, combined):
                F = FS[k]
                (w_, u_, b_, fn) = gorder[gi_]
                gs = st[k].setdefault(f"g{step}", [None] * 4)
                src = (st[k][f"s{step}{gi_}"][:] if combined
                       else st[k]["x"][:, step * F:(step + 1) * F])
                gs[gi_] = pool.tile([NP, F], F16, tag=f"g{gi_}", bufs=2,
                                    name=f"g{step}{gi_}_{k}")
                nc.scalar.activation(gs[gi_][:], src, fn,
                                     bias=float(b_), scale=float(w_))

            def GATES(k, step):
                SG(k, step, 1)                  # DVE combine for g-gate
                if not drop_f:
                    SG(k, step, 2)
                for half in (0, 1):             # i-gate via TE
                    z = ZGEN(k, step, 0, half)
                    GREAD(k, step, 0, half, z)
                GDIRECT(k, step, 1, True)       # g-gate (combined)
                GDIRECT(k, step, 2, not drop_f)  # f-gate
                for half in (0, 1):             # o-gate via TE
                    z = ZGEN(k, step, 3, half)
                    GREAD(k, step, 3, half, z)

            def CH2(k):
                F = FS[k]
                i1, g1, f1, o1 = st[k]["g1"]
                m1 = pool.tile([NP, F], F16, tag="tmA", bufs=2, name=f"m1_{k}")
                nc.vector.tensor_tensor(m1[:], i1[:], g1[:], ALU.mult)
                R = pool.tile([NP, F], F16, tag="tmB", bufs=2, name=f"R_{k}")
                nc.vector.tensor_tensor(R[:], st[k]["c1"][:], f1[:], ALU.mult)
                c2 = pool.tile([NP, F], F16, tag="cc", bufs=2, name=f"c2_{k}")
                nc.vector.tensor_tensor(c2[:], m1[:], R[:], ALU.add)
                st[k]["c2"] = c2

            def TC2(k):
                F = FS[k]
                tc2 = pool.tile([NP, F], F16, tag="tc", bufs=2, name=f"tc2_{k}")
                nc.scalar.activation(tc2[:], st[k]["c2"][:], AF.Tanh,
                                     bias=0.0, scale=1.0)
                st[k]["tc2"] = tc2

            def H2(k):
                F = FS[k]
                h2 = pool.tile([NP, F], F16, tag="h2", bufs=2, name=f"h2_{k}")
                nc.vector.tensor_tensor(h2[:], st[k]["g1"][3][:], st[k]["tc2"][:],
                                        ALU.mult)
                st[k]["h2"] = h2

            def CH3(k):
                F = FS[k]
                i2, g2, f2, _o2 = st[k]["g2"]
                m2 = pool.tile([NP, F], F16, tag="tmA", bufs=2, name=f"m2_{k}")
                nc.vector.tensor_tensor(m2[:], i2[:], g2[:], ALU.mult)
                S = pool.tile([NP, F], F16, tag="tmB", bufs=2, name=f"S_{k}")
                nc.vector.tensor_tensor(S[:], f2[:], st[k]["c2"][:], ALU.mult)
                c3 = pool.tile([NP, F], F16, tag="cc", bufs=2, name=f"c3_{k}")
                nc.vector.tensor_tensor(c3[:], S[:], m2[:], ALU.add)
                st[k]["c3"] = c3

            def TC3(k):
                F = FS[k]
                tc3 = pool.tile([NP, F], F16, tag="tc", bufs=2, name=f"tc3_{k}")
                nc.scalar.activation(tc3[:], st[k]["c3"][:], AF.Tanh,
                                     bias=0.0, scale=1.0)
                st[k]["tc3"] = tc3

            def H3(k):
                F = FS[k]
                h3 = pool.tile([NP, F], F16, tag="h3", bufs=2, name=f"h3_{k}")
                nc.vector.tensor_tensor(h3[:], st[k]["g2"][3][:], st[k]["tc3"][:],
                                        ALU.mult)
                st[k]["h3"] = h3

            def MM(k):
                CHUNKS = _CHK(FS[k])
                pts = []
                for (c0, cw) in CHUNKS:
                    pt = psum_pool.tile([NQ, cw], dt.float32, tag="pt", bufs=4,
                                        name=f"pt_{k}_{c0}")
                    pts.append(pt)
                st[k]["pt"] = pts
                srcs = (st[k]["h1"], st[k]["h2"], st[k]["h3"])
                for ci, (c0, cw) in enumerate(CHUNKS):
                    for ti in range(3):
                        nc.tensor.matmul(pts[ci][:], wt[ti][:],
                                         srcs[ti][:, c0:c0 + cw],
                                         start=(ti == 0), stop=(ti == 2))

            def CP(k, cis=None, act=False):
                F = FS[k]
                CHUNKS = _CHK(F)
                outs = st[k].get("outs")
                if outs is None:
                    outs = pool.tile([NQ, F], F16, tag="outs", bufs=2,
                                     name=f"outs_{k}")
                    st[k]["outs"] = outs
                for ci in (range(len(CHUNKS)) if cis is None else cis):
                    c0, cw = CHUNKS[ci]
                    if act:
                        nc.scalar.activation(outs[:, c0:c0 + cw],
                                             st[k]["pt"][ci][:], AF.Copy,
                                             bias=0.0, scale=1.0)
                    else:
                        nc.vector.tensor_copy(outs[:, c0:c0 + cw],
                                              st[k]["pt"][ci][:])

            def OUT(k):
                nc.sync.dma_start(outd[:, OOFF[k]:OOFF[k] + FS[k]],
                                  st[k]["outs"][:])

            # ---- software pipeline ----
            DMA(0)
            DMA(1)
            T0A(0); T0B(0); T0C(0); T0D(0); T0E(0)
            for k in range(T):
                last = (k == T - 1)
                if k + 2 < T:
                    DMA(k + 2)
                GATES(k, 1)       # TE z1 halves + ACT reads, interleaved
                CH2(k)            # DVE m1, R, c2
                if last and k >= 1:
                    MM(k - 1)     # chunk-outer: chunk 0 completes early
                    CP(k - 1, (0, 1), act=True)   # ACT copies in filler slot
                if k + 1 < T:
                    T0A(k + 1)    # ACT i0,g0 filler while c2 lands
                    T0B(k + 1)    # DVE c1
                TC2(k)            # ACT
                if last and k >= 1:
                    CP(k - 1, (2, 3), act=True)
                    OUT(k - 1)
                if k + 1 < T:
                    T0C(k + 1)    # ACT o0 filler while z2 spins up
                H2(k)             # DVE (unblocks TE z2)
                if k >= 1 and not last:
                    MM(k - 1)     # TE out-linear for previous tile
                GATES(k, 2)       # TE z2 halves + ACT reads
                if k >= 1 and not last:
                    CP(k - 1)     # DVE copies (fill DVE idle before m2)
                    OUT(k - 1)
                CH3(k)            # DVE m2, S, c3
                if k + 1 < T:
                    T0D(k + 1)    # ACT tanh(c1) filler while c3 lands
                TC3(k)            # ACT
                if k + 1 < T:
                    T0E(k + 1)    # DVE h1
                H3(k)
            MM(T - 1)
            CP(T - 1)
            OUT(T - 1)

    nc.finalize()
    return nc


# --------------------------------------------------------------------------
# Safe fallback builder (v2): DVE combines, exact t0, ACT tanh everywhere.
# Correct for arbitrary weights; used when numerical guards fail.
# --------------------------------------------------------------------------
def _build_safe(wkey):
    (wi, wf, wg, wo, ui, uf, ug, uo, bi, bf, bg, bo) = wkey
    import concourse.bacc as bacc
    import concourse.tile as tile
    from concourse import mybir

    dt = mybir.dt
    AF = mybir.ActivationFunctionType
    ALU = mybir.AluOpType
    F16 = dt.float16

    gates = ((wi, ui, bi, AF.Sigmoid),
             (wf, uf, bf, AF.Sigmoid),
             (wg, ug, bg, AF.Tanh),
             (wo, uo, bo, AF.Sigmoid))

    nc = bacc.Bacc("TRN2", target_bir_lowering=False, debug=False)
    for v in sorted({float(b) for (_w, _u, b, _f) in gates}):
        if v == 0.0:
            continue
        t = nc.alloc_sbuf_tensor(f"const-user-{v!r}", [128, 1], dt.float32)
        nc.vector.memset(t.ap(), v)
        nc.const_aps.aps[(dt.float32, v)] = t.ap()
    nc.all_engine_barrier()

    xd = nc.declare_dram_parameter("x", [T, NP, SEQ * F], F16, isOutput=False)
    wds = [nc.declare_dram_parameter(f"w{t + 1}", [NP, NQ], F16, isOutput=False)
           for t in range(3)]
    outd = nc.declare_dram_parameter("out", [T, NQ, F], F16, isOutput=True)

    with tile.TileContext(nc) as tc:
        with tc.tile_pool(name="wpool", bufs=1) as wpool, \
             tc.tile_pool(name="sbuf", bufs=2) as pool, \
             tc.tile_pool(name="psum", bufs=2, space="PSUM") as psum_pool:
            wt = []
            for wd in wds:
                w = wpool.tile([NP, NQ], F16, tag=f"w{wd.name}")
                nc.sync.dma_start(w[:], wd[:])
                wt.append(w)

            def process(k):
                xt = pool.tile([NP, SEQ * F], F16, tag="x", bufs=2, name=f"x_{k}")
                nc.sync.dma_start(xt[:], xd[k])
                x0 = xt[:, 0:F]
                i0 = pool.tile([NP, F], F16, tag="gi", bufs=2, name=f"i0_{k}")
                g0 = pool.tile([NP, F], F16, tag="gg", bufs=2, name=f"g0_{k}")
                o0 = pool.tile([NP, F], F16, tag="go", bufs=2, name=f"o0_{k}")
                nc.scalar.activation(i0[:], x0, AF.Sigmoid, bias=float(bi), scale=float(wi))
                nc.scalar.activation(g0[:], x0, AF.Tanh, bias=float(bg), scale=float(wg))
                nc.scalar.activation(o0[:], x0, AF.Sigmoid, bias=float(bo), scale=float(wo))
                c = pool.tile([NP, F], F16, tag="c", bufs=2, name=f"c1_{k}")
                nc.vector.tensor_tensor(c[:], i0[:], g0[:], ALU.mult)
                tc1 = pool.tile([NP, F], F16, tag="tc", bufs=2, name=f"tc1_{k}")
                nc.scalar.activation(tc1[:], c[:], AF.Tanh, bias=0.0, scale=1.0)
                hs = [None] * 3
                hs[0] = pool.tile([NP, F], F16, tag="h0", bufs=2, name=f"h1_{k}")
                nc.vector.tensor_tensor(hs[0][:], o0[:], tc1[:], ALU.mult)
                for sti in (1, 2):
                    xft = xt[:, sti * F:(sti + 1) * F]
                    hprev = hs[sti - 1]
                    gout = []
                    for gi_, (w_, u_, b_, fn) in enumerate(gates):
                        tmp = pool.tile([NP, F], F16, tag=f"t{gi_}", bufs=2,
                                        name=f"t{gi_}{sti}_{k}")
                        gt = pool.tile([NP, F], F16, tag=f"q{gi_}", bufs=2,
                                       name=f"q{gi_}{sti}_{k}")
                        if abs(u_) > 1e-4:
                            nc.vector.tensor_scalar(tmp[:], xft, float(w_ / u_),
                                                    None, ALU.mult)
                            nc.vector.tensor_tensor(tmp[:], tmp[:], hprev[:], ALU.add)
                            nc.scalar.activation(gt[:], tmp[:], fn, bias=float(b_),
                                                 scale=float(u_))
                        else:
                            nc.vector.tensor_scalar(tmp[:], xft, float(w_),
                                                    None, ALU.mult)
                            nc.scalar.activation(gt[:], tmp[:], fn, bias=float(b_),
                                                 scale=1.0)
                        gout.append(gt)
                    m1 = pool.tile([NP, F], F16, tag="m1", bufs=2, name=f"m1{sti}_{k}")
                    m2 = pool.tile([NP, F], F16, tag="m2", bufs=2, name=f"m2{sti}_{k}")
                    nc.vector.tensor_tensor(m1[:], gout[0][:], gout[2][:], ALU.mult)
                    nc.vector.tensor_tensor(m2[:], gout[1][:], c[:], ALU.mult)
                    c = pool.tile([NP, F], F16, tag="c", bufs=2, name=f"c{sti + 1}_{k}")
                    nc.vector.tensor_tensor(c[:], m1[:], m2[:], ALU.add)
                    tct = pool.tile([NP, F], F16, tag="tc", bufs=2, name=f"tc{sti + 1}_{k}")
                    nc.scalar.activation(tct[:], c[:], AF.Tanh, bias=0.0, scale=1.0)
                    hs[sti] = pool.tile([NP, F], F16, tag=f"h{sti}", bufs=2,
                                        name=f"h{sti + 1}_{k}")
                    nc.vector.tensor_tensor(hs[sti][:], gout[3][:], tct[:], ALU.mult)
                pts = []
                for (c0, cw) in CHUNKS:
                    pt = psum_pool.tile([NQ, cw], dt.float32, tag="pt", bufs=8,
                                        name=f"pt_{k}_{c0}")
                    pts.append(pt)
                for ti in range(3):
                    for ci, (c0, cw) in enumerate(CHUNKS):
                        nc.tensor.matmul(pts[ci][:], wt[ti][:], hs[ti][:, c0:c0 + cw],
                                         start=(ti == 0), stop=(ti == 2))
                outs = pool.tile([NQ, F], F16, tag="outs", bufs=2, name=f"outs_{k}")
                for ci, (c0, cw) in enumerate(CHUNKS):
                    nc.vector.tensor_copy(outs[:, c0:c0 + cw], pts[ci][:])
                nc.sync.dma_start(outd[k], outs[:])

            for k in range(T):
                process(k)

    nc.finalize()
    return nc


def kernel(x, w_ih, w_hh, b_ih, b_hh, w_lin, b_lin):
    from concourse.bass_utils import run_bass_kernel_spmd

    x = np.asarray(x, dtype=np.float32)
    w_ih = np.asarray(w_ih, dtype=np.float32)
    w_hh = np.asarray(w_hh, dtype=np.float32)
    b_ih = np.asarray(b_ih, dtype=np.float32)
    b_hh = np.asarray(b_hh, dtype=np.float32)
    w_lin = np.asarray(w_lin, dtype=np.float32)
    b_lin = np.asarray(b_lin, dtype=np.float32)

    wi, wf, wg, wo = (float(v) for v in w_ih[:, 0])
    ui, uf, ug, uo = (float(v) for v in w_hh[:, 0])
    bias = b_ih + b_hh
    bi, bf, bg, bo = (float(v) for v in bias)
    wl = w_lin[0].astype(np.float64)          # [18]
    bl = float(b_lin[0])

    wkey = (wi, wf, wg, wo, ui, uf, ug, uo, bi, bf, bg, bo)
    if wkey not in _CACHE:
        v4_ok = all(1e-3 < abs(w_) and abs(u_ / w_) < 1e4 for w_, u_ in
                    ((wi, ui), (wf, uf), (wg, ug), (wo, uo)))
        if v4_ok:
            # tc1 odd-poly fit over the analytic |c1| <= max|i0*g0| range
            xs = np.linspace(-6.5, 6.5, 4001)
            c1g = (1.0 / (1.0 + np.exp(-(wi * xs + bi)))) * np.tanh(wg * xs + bg)
            rmax = min(1.0, np.abs(c1g).max() * 1.12 + 0.02)
            gr = np.linspace(-rmax, rmax, 2001)
            A = np.stack([gr, gr ** 3, gr ** 5], 1)
            coef, *_ = np.linalg.lstsq(A, np.tanh(gr), rcond=None)
            err = np.abs(A @ coef - np.tanh(gr)).max()
            qs1 = tuple(float(v) for v in coef) if err < 2.5e-3 else None
            drop_f = bool(abs(uf) <= 0.02)
            key = (wkey, qs1, drop_f)
            _CACHE[wkey] = ("v4", _build_v4(key))
        else:
            _CACHE[wkey] = ("safe", _build_safe(wkey))
    path, nc = _CACHE[wkey]

    # Linear stationaries: W_t[p, q] = wl[3*(p%6) + t] if q == p//6.
    p = np.arange(NP)
    wmats = []
    for t in range(3):
        W = np.zeros((NP, NQ), dtype=np.float16)
        W[p, p // 6] = wl[3 * (p % 6) + t].astype(np.float16)
        wmats.append(W)
    bl_tot = np.float32(bl)

    # z-combine weights: identity + diag(u_g/w_g), gate order (i,g,f,o).
    extra = {}
    if path == "v4":
        zw = np.zeros((5, NP, NP), dtype=np.float16)
        zw[0][p, p] = 1.0
        go = ((wi, ui), (wg, ug), (wf, uf), (wo, uo))
        for gi_, (w_, u_) in enumerate(go):
            zw[1 + gi_][p, p] = np.float16(u_ / w_)
        extra["zw"] = zw

    # Host data prep: [B, 3, 1] -> per-core partition-major fp16.
    xb = x.reshape(B, SEQ).astype(np.float16)
    in_maps = []
    for c in range(N_CORES):
        xc = xb[c * Bc:(c + 1) * Bc]
        if path == "v4":
            # variable-F: group g = NQ*OOFF[k] + q*FS[k] + j
            xp = np.zeros((PAD_E2, SEQ), dtype=np.float16)
            xp[:Bc] = xc
            xr = np.empty((NP, 3 * SUMF), dtype=np.float16)
            for k in range(T):
                Fk = FS[k]
                e0 = 6 * NQ * OOFF[k]
                blk = xp[e0:e0 + 6 * NQ * Fk]
                # [q, j, m, t] -> [q, m, t, j]
                b4 = blk.reshape(NQ, Fk, 6, SEQ).transpose(0, 2, 3, 1)
                xr[:, XOFF[k]:XOFF[k] + 3 * Fk] =                     np.ascontiguousarray(b4).reshape(NP, 3 * Fk)
        else:
            xp = np.zeros((PAD_E, SEQ), dtype=np.float16)
            xp[:Bc] = xc
            xr = xp.reshape(T, NQ, F, 6, SEQ).transpose(0, 1, 3, 4, 2)
            xr = np.ascontiguousarray(xr).reshape(T, NP, SEQ * F)
        im = {"x": xr, "w1": wmats[0], "w2": wmats[1], "w3": wmats[2]}
        im.update(extra)
        in_maps.append(im)

    res = run_bass_kernel_spmd(nc, in_maps, list(range(N_CORES)))

    out = np.empty((B // 6, 1), dtype=np.float32)
    for c in range(N_CORES):
        oc = res.results[c]["out"]
        if path == "v4":
            # out[q, OOFF[k]+j] -> group NQ*OOFF[k] + q*FS[k] + j
            flat = np.empty(NQ * SUMF, dtype=np.float16)
            for k in range(T):
                Fk = FS[k]
                flat[NQ * OOFF[k]:NQ * (OOFF[k] + Fk)] =                     oc[:, OOFF[k]:OOFF[k] + Fk].ravel()
            oc = flat
        oc = oc.reshape(-1)[:GC].astype(np.float32)
        out[c * GC:(c + 1) * GC, 0] = oc + bl_tot
    return out
